# revision 22
# baseline (speedup 1.0000x reference)
"""Trainium2 Bass kernel for nn_Attention_33036888441230.

Cross-attention transformer block, B=8 batch sharded 1-per-core across 8
NeuronCores (pure data parallel, no collectives).

reference math (per batch):
  xn = LN(x,g1,b1); yn = LN(y,g2,b2)
  q = (xn@wq+bq).view(N,H,64); k = (yn@wk+bk).view(M,H,64)
  v = (yn@wv+bv).view(M,H,E)
  a = softmax(q.k^T/8, axis=m)
  dx = einsum('hnm,mhe->ne', a, v)       # heads summed
  h = LN3(xn + dx); out1 = h + relu(h@w_in+b_in)@w_out + b_out
  returns (out1, yn)

Implementation notes:
  - all large matmuls (q/k/v projections, attention.V, both MLP matmuls) run
    in fp8e4 with perf_mode=DoubleRow (2 fp8 weights per PE cell, ~2x rate);
    contraction pairs live in [P, 2, N] tiles (pair = two 128-deep k-subtiles
    adjacent in the free dim, matching the DoubleRow ISA layout).
  - weights are scaled by powers of two on host so fp8e4 sees ~unit-range
    values (TRN e4m3: max 240, min normal 2^-6); the inverse scales fold into
    the existing PSUM-evacuation activations (scale=) and, for the attention
    value path, into the ones-column (ones=32 == wv scale) so the softmax
    normalization ratio is exact and needs no extra ops.
  - activation transposes (xn^T, yn^T, h^T feature-major for the PE's
    contraction-on-partition) run on the DMA xbar transpose engine from bf16
    copies, then a DVE cast to fp8 -- zero PE time, vs ~45us of PE transposes.
  - softmax has no max-subtraction (scores provably tiny); row-sums come for
    free from a 32.0-column appended to the attention.V matmul; 1/rowsum is
    fused into the per-head accumulate (scalar_tensor_tensor).
  - softmax-of-rows sums to 1 => v-bias contributes sum_h bv_h; that row is
    broadcast once and pre-folded (together with the xn residual) into the
    attention accumulator during phase A, off the critical tail.
  - phase A interleaves per-tile layer-norms with projection matmul tiles so
    the PE never drains; the LN3 stats/normalize for the MLP run on ScalarE
    (Identity/Square + accum_out) woven under the last head pair's attention
    matmuls -- nothing but DoubleRow matmuls touches the PE there.
  - token-major for attention-weighted sums; scores in bf16; softmax
    normalization / residual accumulate in fp32.
  - score matmuls: heads in pairs, even head in PE rows 0:64, odd head in
    rows 64:128, interleaved so both run concurrently in the array.
"""

import sys

if "/opt/trn_rl_repo" not in sys.path:
    sys.path.insert(0, "/opt/trn_rl_repo")

from contextlib import ExitStack

import numpy as np

import concourse.bass as bass
import concourse.mybir as mybir
import concourse.tile as tile
from concourse import bacc
from concourse.masks import make_identity

F32 = mybir.dt.float32
BF16 = mybir.dt.bfloat16
F8 = mybir.dt.float8e4
OP = mybir.AluOpType
AF = mybir.ActivationFunctionType
DR = mybir.MatmulPerfMode.DoubleRow

P = 128
HD = 64
EPS = 1e-5
N_CORES = 8

# host-side power-of-two scales folded into fp8 weights
S_Q = 512.0   # wq (already /8 for attn scale) -> fp8
S_K = 64.0    # wk -> fp8
S_V = 32.0    # wv -> fp8; also the value of the row-sum ones-column
S_WI = 64.0   # w_in -> fp8
S_WO = 64.0   # w_out -> fp8 (and b_out pre-scaled by S_WO on host)

VP = 784      # v tile slot stride: 768 cols of v + ones col at 768, pad to 784


def _chunks(n, c=512):
    return [(i, min(i + c, n)) for i in range(0, n, c)]


def build(T, E, H, aff=(False, False, False)):
    """Build the per-core Bass graph. T tokens, E embed, H heads (HD=64)."""
    TT, ET = T // P, E // P
    NG = ET // 2   # contraction pair-groups for an E-deep reduction
    MG = TT // 2   # contraction pair-groups for a T-deep reduction
    HT = TT // 2   # token tiles per half
    assert H * HD == E
    nc = bacc.Bacc("TRN2", num_devices=N_CORES)

    x_d = nc.dram_tensor("x", [T, E], F32, kind="ExternalInput").ap()
    y_d = nc.dram_tensor("y", [T, E], F32, kind="ExternalInput").ap()
    wq_d = nc.dram_tensor("wq", [E, E], F8, kind="ExternalInput").ap()
    wk_d = nc.dram_tensor("wk", [E, E], F8, kind="ExternalInput").ap()
    wv_d = nc.dram_tensor("wv", [H, E, E], F8, kind="ExternalInput").ap()
    wi_d = nc.dram_tensor("w_in", [E, E], F8, kind="ExternalInput").ap()
    wo_d = nc.dram_tensor("w_out", [E, E], F8, kind="ExternalInput").ap()
    bq_d = nc.dram_tensor("bq", [E], F32, kind="ExternalInput").ap()
    bk_d = nc.dram_tensor("bk", [E], F32, kind="ExternalInput").ap()
    bvs_d = nc.dram_tensor("bvs", [E], BF16, kind="ExternalInput").ap()
    bi_d = nc.dram_tensor("b_in", [E], F32, kind="ExternalInput").ap()
    bo_d = nc.dram_tensor("b_out", [E], BF16, kind="ExternalInput").ap()
    aff_d = {}
    for i, need in enumerate(aff):
        if need:
            aff_d[i] = (
                nc.dram_tensor(f"affg{i}", [E], BF16, kind="ExternalInput").ap(),
                nc.dram_tensor(f"affb{i}", [E], BF16, kind="ExternalInput").ap(),
            )
    o1_d = nc.dram_tensor("o1", [T, E], F32, kind="ExternalOutput").ap()
    oyn_d = nc.dram_tensor("oyn", [T, E], F32, kind="ExternalOutput").ap()

    with tile.TileContext(nc) as tc, ExitStack() as ctx:
        persist = ctx.enter_context(tc.tile_pool(name="persist", bufs=1))
        ps_big = ctx.enter_context(tc.tile_pool(name="psb", bufs=2, space="PSUM"))
        stp = ctx.enter_context(tc.tile_pool(name="stats", bufs=8))
        trp = ctx.enter_context(tc.tile_pool(name="trp", bufs=6))

        acc = persist.tile([P, TT, E], F32, tag="acc")
        ynT8 = persist.tile([P, ET, T], F8, tag="ynT8")
        BV = persist.tile([P, E], F32, tag="BV")
        ident = persist.tile([P, P], F32, tag="ident")
        ones_r = persist.tile([1, P], BF16, tag="ones")
        bq_sb = persist.tile([P, ET], F32, tag="bq")
        bk_sb = persist.tile([P, ET], F32, tag="bk")
        bi_sb = persist.tile([P, ET], F32, tag="bi")
        bo_row = persist.tile([1, E], BF16, tag="bo")
        bvs_row = persist.tile([1, E], BF16, tag="bvs")
        eps_col = persist.tile([P, 1], F32, tag="eps")
        iswo_col = persist.tile([P, 1], F32, tag="iswo")

        make_identity(nc, ident[:])
        nc.vector.memset(ones_r[:], 1.0)
        nc.vector.memset(eps_col[:], EPS)
        nc.vector.memset(iswo_col[:], 1.0 / S_WO)
        nc.sync.dma_start(bq_sb[:], bq_d.rearrange("(a p) -> p a", p=P))
        nc.sync.dma_start(bk_sb[:], bk_d.rearrange("(a p) -> p a", p=P))
        nc.sync.dma_start(bi_sb[:], bi_d.rearrange("(a p) -> p a", p=P))
        nc.sync.dma_start(bo_row[:], bo_d[None, :])
        nc.sync.dma_start(bvs_row[:], bvs_d[None, :])

        def bcast_row(row_ap, dst):  # (1,E) -> (128,E) via K=1 matmul
            ps = ps_big.tile([P, 1024], F32, tag="big")
            for c0, c1 in _chunks(E):
                nc.tensor.matmul(
                    ps[:, c0:c1], ones_r[:], row_ap[:, c0:c1],
                    start=True, stop=True,
                )
            nc.vector.tensor_copy(dst[:], ps[:, :E])

        aff_sb = {}
        for i, (gd, bd) in aff_d.items():
            g_row = persist.tile([1, E], BF16, tag=f"agr{i}")
            b_row = persist.tile([1, E], BF16, tag=f"abr{i}")
            nc.sync.dma_start(g_row[:], gd[None, :])
            nc.sync.dma_start(b_row[:], bd[None, :])
            g_t = persist.tile([P, E], F32, tag=f"ag{i}")
            b_t = persist.tile([P, E], F32, tag=f"ab{i}")
            bcast_row(g_row, g_t)
            bcast_row(b_row, b_t)
            aff_sb[i] = (g_t, b_t)

        qkp = ctx.enter_context(tc.tile_pool(name="qk", bufs=1))
        qT = qkp.tile([P, ET, T], BF16, tag="qT")
        kT = qkp.tile([P, ET, T], BF16, tag="kT")

        def layer_norm_tile(dst_ap, src_ap):
            """dst = (src - mean)/sqrt(var+eps), per-partition stats over E."""
            st6 = stp.tile([P, 2, 6], F32, tag="st6")
            half = src_ap.shape[-1] // 2
            nc.vector.bn_stats(st6[:, 0, :], src_ap[:, :half])
            nc.vector.bn_stats(st6[:, 1, :], src_ap[:, half:])
            mv = stp.tile([P, 2], F32, tag="mv")
            nc.vector.bn_aggr(mv[:], st6[:])
            std = stp.tile([P, 1], F32, tag="std")
            nc.scalar.activation(std[:], mv[:, 1:2], AF.Sqrt, bias=eps_col[:])
            rst = stp.tile([P, 1], F32, tag="rst")
            nc.vector.reciprocal(rst[:], std[:])
            nmr = stp.tile([P, 1], F32, tag="nmr")
            nc.vector.tensor_scalar(
                nmr[:], mv[:, 0:1], rst[:], -1.0, op0=OP.mult, op1=OP.mult
            )
            # big apply pass on ACT (idle at startup): x*rstd + (-mu*rstd)
            nc.scalar.activation(
                dst_ap, src_ap, AF.Identity, bias=nmr[:], scale=rst[:]
            )

        def affine_tile(dst_ap, src_ap, idx):
            g_t, b_t = aff_sb[idx]
            nc.vector.tensor_mul(dst_ap, src_ap, g_t[:])
            nc.vector.tensor_add(dst_ap, dst_ap, b_t[:])

        # one [P, E] token-tile transposed to feature-major in a single DMA
        # xbar op (no PE time), alternating the two HWDGE queues; the bf16
        # staging tile then casts to fp8 into the [P, ET, T] destination
        def dmaT_tile(dstT8, src_row, tt, q):
            ytt = trp.tile([P, ET, P], BF16, tag="ytt")
            eng = nc.sync if q % 2 == 0 else nc.scalar
            eng.dma_start_transpose(ytt[:], src_row)
            nc.vector.tensor_copy(
                dstT8[:, :, tt * P:(tt + 1) * P], ytt[:]
            )

        # v-projection machinery (used from phase A for the first head pair
        # and from each pair's score front for the NEXT pair: the pairs are
        # software-pipelined one stage ahead so the PE always has DoubleRow
        # work while ACT produces exp tiles / DVE runs layer-norm chains)
        wvp = ctx.enter_context(tc.tile_pool(name="wvp", bufs=4))
        vpp = ctx.enter_context(tc.tile_pool(name="vp", bufs=17))
        wv8_sb = {}

        def load_wv(h):
            wt = wvp.tile([P, ET, E], F8, tag="wv")
            nc.gpsimd.dma_start(
                wt[:], wv_d[h].rearrange("(a p) e -> p a e", p=P)
            )
            wv8_sb[h] = wt

        # v-projection for one m-tile, both heads of a pair, one shared
        # stationary (ynT8 tokens) per contraction pair-group
        def vproj_mt(h0, h1, mt, vt):
            sl = mt % 2
            if sl == 0:
                for h in (h0, h1):
                    v_ = vpp.tile([P, 2, VP], F8, tag="v")
                    nc.gpsimd.memset(v_[:, :, E:E + 1], S_V)
                    vt[h].append(v_)
            psv0 = ps_big.tile([P, 1024], F32, tag="big")
            psv1 = ps_big.tile([P, 1024], F32, tag="big")
            for g in range(NG):
                lhs = ynT8[:, 2 * g:2 * g + 2, mt * P:(mt + 1) * P]
                for c0, c1 in _chunks(E):
                    nc.tensor.matmul(
                        psv0[:, c0:c1], lhs,
                        wv8_sb[h0][:, 2 * g:2 * g + 2, c0:c1],
                        start=(g == 0), stop=(g == NG - 1),
                        perf_mode=DR,
                    )
                    nc.tensor.matmul(
                        psv1[:, c0:c1], lhs,
                        wv8_sb[h1][:, 2 * g:2 * g + 2, c0:c1],
                        start=(g == 0), stop=(g == NG - 1),
                        perf_mode=DR,
                    )
            nc.vector.tensor_copy(vt[h0][mt // 2][:, sl, :E], psv0[:, :E])
            nc.vector.tensor_copy(vt[h1][mt // 2][:, sl, :E], psv1[:, :E])

        with tc.tile_pool(name="bc1", bufs=1) as bcp, \
             tc.tile_pool(name="io3", bufs=9) as iop, \
             tc.tile_pool(name="iob", bufs=4) as iob, \
             tc.tile_pool(name="wpr", bufs=2) as wp, \
             tc.tile_pool(name="pstr", bufs=2, space="PSUM") as ps_tr:
            yn = bcp.tile([P, TT, E], F32, tag="yn")
            xnT8 = bcp.tile([P, ET, T], F8, tag="xnT8")

            # ~5us of junk matmuls on the identity tile: spins the PE HAM
            # activity window to full clock while the first input DMAs and
            # layer-norms run, so real matmuls start at 2.4 GHz
            warm = ps_tr.tile([P, 2, P], F32, tag="tr")
            for _ in range(48):
                nc.tensor.matmul(warm[:, 0, :], ident[:], ident[:],
                                 start=True, stop=True)

            # layer-norm stats only (DVE + one tiny ACT sqrt), no apply;
            # returns the (rstd, -mu*rstd) columns for a later apply pass
            def ln_stats(src_ap):
                st6 = stp.tile([P, 2, 6], F32, tag="st6")
                half = src_ap.shape[-1] // 2
                nc.vector.bn_stats(st6[:, 0, :], src_ap[:, :half])
                nc.vector.bn_stats(st6[:, 1, :], src_ap[:, half:])
                mv = stp.tile([P, 2], F32, tag="mv")
                nc.vector.bn_aggr(mv[:], st6[:])
                std = stp.tile([P, 1], F32, tag="std")
                nc.scalar.activation(std[:], mv[:, 1:2], AF.Sqrt,
                                     bias=eps_col[:])
                rst = stp.tile([P, 1], F32, tag="rst")
                nc.vector.reciprocal(rst[:], std[:])
                nmr = stp.tile([P, 1], F32, tag="nmr")
                nc.vector.tensor_scalar(
                    nmr[:], mv[:, 0:1], rst[:], -1.0, op0=OP.mult, op1=OP.mult
                )
                return rst, nmr

            def ln_y_tail(tt, it, rst, nmr):
                nc.scalar.activation(yn[:, tt, :], it[:], AF.Identity,
                                     bias=nmr[:], scale=rst[:])
                if 1 in aff_sb:
                    ya = iop.tile([P, E], F32, tag="yaff")
                    affine_tile(ya[:], yn[:, tt, :], 1)
                    nc.gpsimd.dma_start(oyn_d[tt * P:(tt + 1) * P, :], ya[:])
                    src = ya[:]
                else:
                    nc.gpsimd.dma_start(
                        oyn_d[tt * P:(tt + 1) * P, :], yn[:, tt, :]
                    )
                    src = yn[:, tt, :]
                ynb = iob.tile([P, E], BF16, tag="ynb")
                nc.scalar.activation(ynb[:], src, AF.Identity)
                dmaT_tile(ynT8, ynb[:], tt, tt)

            # layer-norm x straight to bf16, then pre-fold the attention
            # accumulator acc = xn + sum_h bv_h (residual add off the tail)
            def ln_x_tail(tt, it, rst, nmr):
                xnb = iob.tile([P, E], BF16, tag="xnb")
                if 0 in aff_sb:
                    xt = iop.tile([P, E], F32, tag="xaff")
                    nc.scalar.activation(xt[:], it[:], AF.Identity,
                                         bias=nmr[:], scale=rst[:])
                    affine_tile(xnb[:], xt[:], 0)
                else:
                    nc.scalar.activation(xnb[:], it[:], AF.Identity,
                                         bias=nmr[:], scale=rst[:])
                nc.vector.tensor_add(acc[:, tt, :], xnb[:], BV[:])
                dmaT_tile(xnT8, xnb[:], tt, tt)

            def load_w8(w_d):
                wt = wp.tile([P, ET, E], F8, tag="w")
                nc.gpsimd.dma_start(
                    wt[:], w_d.rearrange("(a p) e -> p a e", p=P)
                )
                return wt

            # one output-feature tile of an fp8 DoubleRow projection
            def proj_mt(w8, b_sb, outT, srcT8, mt, c0, c1, scale):
                ps = ps_big.tile([P, 1024], F32, tag="big")
                for g in range(NG):
                    nc.tensor.matmul(
                        ps[:, : c1 - c0],
                        w8[:, 2 * g:2 * g + 2, mt * P:(mt + 1) * P],
                        srcT8[:, 2 * g:2 * g + 2, c0:c1],
                        start=(g == 0), stop=(g == NG - 1),
                        perf_mode=DR,
                    )
                nc.scalar.activation(
                    outT[:, mt, c0:c1], ps[:, : c1 - c0], AF.Identity,
                    bias=b_sb[:, mt:mt + 1], scale=scale,
                )

            # weight DMAs + BV broadcast first so everything is on-chip
            wk8 = load_w8(wk_d)
            wq8 = load_w8(wq_d)
            load_wv(0)
            load_wv(1)
            bcast_row(bvs_row, BV)
            # phase A is emitted op-type-batched so every engine FIFO runs a
            # clean stream (input DMAs -> all LN stats -> per-tile
            # apply/transpose/cast chains) with matmul bursts woven in:
            # the y tiles host head pair 0's v-projection (a token tile's
            # ynT8 column is ready right after its own transpose+cast),
            # the x tiles host the k/q projection feature-tiles
            ity = []
            for tt in range(TT):
                ity.append(iop.tile([P, E], F32, tag="in", name=f"iy{tt}"))
                nc.sync.dma_start(ity[tt][:], y_d[tt * P:(tt + 1) * P, :])
            sny = [ln_stats(ity[tt][:]) for tt in range(TT)]
            vt0 = {0: [], 1: []}
            for tt in range(TT):
                ln_y_tail(tt, ity[tt], *sny[tt])
                vproj_mt(0, 1, tt, vt0)
            itx = []
            for tt in range(TT):
                itx.append(iop.tile([P, E], F32, tag="in", name=f"ix{tt}"))
                nc.sync.dma_start(itx[tt][:], x_d[tt * P:(tt + 1) * P, :])
            snx = [ln_stats(itx[tt][:]) for tt in range(TT)]
            # k-projection + first-half q: 18 bursts over the 8 x tiles
            kcalls = [(mt, hf) for mt in range(ET) for hf in (0, 1)]
            qh1 = [[], [], [], [], [0, 1], [2, 3], [4], [5]]
            for tt in range(TT):
                ln_x_tail(tt, itx[tt], *snx[tt])
                if tt < 4:
                    for mt, hf in kcalls[tt * 3:tt * 3 + 3]:
                        proj_mt(wk8, bk_sb, kT, ynT8, mt,
                                hf * (T // 2), (hf + 1) * (T // 2), 1.0 / S_K)
                for mt in qh1[tt]:
                    proj_mt(wq8, bq_sb, qT, xnT8, mt, 0, T // 2, 1.0 / S_Q)
            for mt in range(ET):
                proj_mt(wq8, bq_sb, qT, xnT8, mt, T // 2, T, 1.0 / S_Q)

        # --- attention head loop (LN3+MLP overlapped under the last pair) ---
        with tc.tile_pool(name="expp", bufs=17) as expp, \
             tc.tile_pool(name="rcp", bufs=6) as rcp:
            sc_stack = ExitStack()
            ps_sc = sc_stack.enter_context(
                tc.tile_pool(name="pssc", bufs=2, space="PSUM")
            )

            # one head x one token-tile of attention.V (fp8 DoubleRow);
            # ps[:, E] = S_V * rowsum -> fused normalize-accumulate
            def t_nt(nt, expt, vt):
                ps = ps_big.tile([P, 1024], F32, tag="big")
                for g in range(MG):
                    for c0, c1 in _chunks(E + 1):
                        nc.tensor.matmul(
                            ps[:, c0:c1],
                            expt[g][:, :, nt * P:(nt + 1) * P],
                            vt[g][:, :, c0:c1],
                            start=(g == 0), stop=(g == MG - 1),
                            perf_mode=DR,
                        )
                rc = rcp.tile([P, 1], F32, tag="rc")
                nc.vector.reciprocal(rc[:], ps[:, E:E + 1])
                nc.vector.scalar_tensor_tensor(
                    acc[:, nt, :], ps[:, :E], rc[:], acc[:, nt, :],
                    op0=OP.mult, op1=OP.add,
                )

            def t_phase(h, expt, vt):
                for nt in range(TT):
                    t_nt(nt, expt, vt)

            # heads in pairs: even head uses PE rows 0:64, odd head rows
            # 64:128 -> interleaved score matmuls run concurrently in the
            # array (distinct row groups). The NEXT pair's v-projection
            # (one shared ynT8 weight-load per (mt, g)) is woven between
            # score mt-steps so the PE never waits on the ACT exp drain:
            # pair j's v was already projected during pair j-1's front
            # (pair 0's during phase A).
            def pair_front(j, vt_next):
                h0, h1 = 2 * j, 2 * j + 1
                if vt_next is not None:
                    load_wv(h0 + 2)
                    load_wv(h1 + 2)
                expt = {h0: [], h1: []}
                for mt in range(TT):
                    sl = mt % 2
                    if sl == 0:
                        for h in (h0, h1):
                            ex = expp.tile([P, 2, T], F8, tag="exp")
                            expt[h].append(ex)
                    ps0 = ps_sc.tile([P, 1024], F32, tag="sc")
                    ps1 = ps_sc.tile([P, 1024], F32, tag="sc")
                    for c0, c1 in _chunks(T):
                        nc.tensor.matmul(
                            ps0[:, c0:c1],
                            kT[0:HD, j, mt * P:(mt + 1) * P],
                            qT[0:HD, j, c0:c1],
                            start=True, stop=True,
                        )
                        nc.tensor.matmul(
                            ps1[:, c0:c1],
                            kT[HD:P, j, mt * P:(mt + 1) * P],
                            qT[HD:P, j, c0:c1],
                            start=True, stop=True,
                        )
                    for h, ps in ((h0, ps0), (h1, ps1)):
                        nc.scalar.activation(
                            expt[h][mt // 2][:, sl, :], ps[:, :T], AF.Exp
                        )
                    if vt_next is not None:
                        vproj_mt(h0 + 2, h1 + 2, mt, vt_next)
                return expt

            vt_cur = vt0
            for j in range(H // 2 - 1):
                h0, h1 = 2 * j, 2 * j + 1
                vt_next = {h0 + 2: [], h1 + 2: []}
                expt = pair_front(j, vt_next)
                if j == H // 2 - 2:
                    # emit the last pair's scores early: its 16 exp
                    # activations hide under the previous pair's attention
                    expt_l = pair_front(H // 2 - 1, None)
                t_phase(h0, expt[h0], vt_cur[h0])
                t_phase(h1, expt[h1], vt_cur[h1])
                vt_cur = vt_next
            hl0, hl1 = H - 2, H - 1
            expt = expt_l
            vt = vt_cur
            sc_stack.close()

            # --- residual + LN3 + MLP (fp8 DoubleRow) ---
            with tc.tile_pool(name="mlp", bufs=1) as mp, \
                 tc.tile_pool(name="out3", bufs=3) as op_, \
                 tc.tile_pool(name="scrp", bufs=2) as scrp:
                hT8 = mp.tile([P, ET, T], F8, tag="hT8")
                ru8T = mp.tile([P, ET, T], F8, tag="ru8T")
                wi8 = mp.tile([P, ET, E], F8, tag="wi")
                wo8 = mp.tile([P, ET, E], F8, tag="wo")
                hb = mp.tile([P, TT, E], BF16, tag="hb")
                nc.gpsimd.dma_start(
                    wi8[:], wi_d.rearrange("(a p) l -> p a l", p=P)
                )
                nc.gpsimd.dma_start(
                    wo8[:], wo_d.rearrange("(a p) l -> p a l", p=P)
                )

                # LN3 stats + normalize entirely on ScalarE/few tiny DVE ops
                # (no PE work, no bn_stats -- DVE is busy with the attention
                # evacuations in this window); normalized h lands in bf16 hb
                def ln3(nt):
                    s1 = stp.tile([P, 1], F32, tag="s1")
                    sq = stp.tile([P, 1], F32, tag="sq")
                    scr = scrp.tile([P, E], F32, tag="scr")
                    nc.scalar.activation(
                        scr[:], acc[:, nt, :], AF.Identity, accum_out=s1[:]
                    )
                    nc.scalar.activation(
                        scr[:], acc[:, nt, :], AF.Square, accum_out=sq[:]
                    )
                    m2 = stp.tile([P, 1], F32, tag="m2")
                    nc.scalar.activation(m2[:], s1[:], AF.Square, scale=1.0 / E)
                    vpe = stp.tile([P, 1], F32, tag="vpe")
                    nc.vector.tensor_scalar(
                        vpe[:], sq[:], 1.0 / E, m2[:],
                        op0=OP.mult, op1=OP.subtract,
                    )
                    std = stp.tile([P, 1], F32, tag="std")
                    nc.scalar.activation(std[:], vpe[:], AF.Sqrt, bias=eps_col[:])
                    rst = stp.tile([P, 1], F32, tag="rst")
                    nc.vector.reciprocal(rst[:], std[:])
                    nmr = stp.tile([P, 1], F32, tag="nmr")
                    nc.vector.tensor_scalar(
                        nmr[:], s1[:], rst[:], -1.0 / E, op0=OP.mult, op1=OP.mult
                    )
                    if 2 in aff_sb:
                        ha = scrp.tile([P, E], F32, tag="ha")
                        nc.scalar.activation(
                            ha[:], acc[:, nt, :], AF.Identity,
                            bias=nmr[:], scale=rst[:],
                        )
                        affine_tile(hb[:, nt, :], ha[:], 2)
                    else:
                        nc.scalar.activation(
                            hb[:, nt, :], acc[:, nt, :], AF.Identity,
                            bias=nmr[:], scale=rst[:],
                        )

                # last pair: both heads' attention interleaved per token
                # tile, the LN3 chain and h^T DMA transposes hidden under
                # the PE matmuls
                for nt in range(TT):
                    t_nt(nt, expt[hl0], vt[hl0])
                    t_nt(nt, expt[hl1], vt[hl1])
                    ln3(nt)
                    dmaT_tile(hT8, hb[:, nt, :], nt, nt)

                def u_chunk(c0, c1):
                    # u^T = relu(w_in^T @ hT8 / S_WI + b_in), token columns
                    for mt in range(ET):
                        ps = ps_big.tile([P, 1024], F32, tag="big")
                        for g in range(NG):
                            nc.tensor.matmul(
                                ps[:, : c1 - c0],
                                wi8[:, 2 * g:2 * g + 2, mt * P:(mt + 1) * P],
                                hT8[:, 2 * g:2 * g + 2, c0:c1],
                                start=(g == 0), stop=(g == NG - 1),
                                perf_mode=DR,
                            )
                        nc.scalar.activation(
                            ru8T[:, mt, c0:c1], ps[:, : c1 - c0], AF.Relu,
                            bias=bi_sb[:, mt:mt + 1], scale=1.0 / S_WI,
                        )

                def out_tile(nt):
                    # out1 = (ru8T^T @ wo8 + S_WO*b_out)/S_WO + h
                    ps = ps_big.tile([P, 1024], F32, tag="big")
                    for g in range(NG):
                        for c0, c1 in _chunks(E):
                            nc.tensor.matmul(
                                ps[:, c0:c1],
                                ru8T[:, 2 * g:2 * g + 2, nt * P:(nt + 1) * P],
                                wo8[:, 2 * g:2 * g + 2, c0:c1],
                                start=(g == 0), stop=False,
                                perf_mode=DR,
                            )
                    for c0, c1 in _chunks(E):
                        nc.tensor.matmul(
                            ps[:, c0:c1], ones_r[:], bo_row[:, c0:c1],
                            start=False, stop=True,
                        )
                    ot = op_.tile([P, E], F32, tag="ot")
                    nc.vector.scalar_tensor_tensor(
                        ot[:], ps[:, :E], iswo_col[:], hb[:, nt, :],
                        op0=OP.mult, op1=OP.add,
                    )
                    nc.gpsimd.dma_start(o1_d[nt * P:(nt + 1) * P, :], ot[:])

                u_chunk(0, T // 2)
                for nt in range(TT // 2):
                    out_tile(nt)
                u_chunk(T // 2, T)
                for nt in range(TT // 2, TT):
                    out_tile(nt)

    return nc


def host_prep(inputs, T, E, H):
    """Fold LN affines / scale / v-bias into weights (float64 on host)."""
    f8 = {k: np.asarray(v, np.float64) for k, v in inputs.items()}
    g1, b1 = f8["ln1_g"], f8["ln1_b"]
    g2, b2 = f8["ln2_g"], f8["ln2_b"]
    g3, b3 = f8["ln3_g"], f8["ln3_b"]
    scale = 1.0 / np.sqrt(HD)
    wq_f = (g1[:, None] * f8["wq"]) * scale
    bq_f = (b1 @ f8["wq"] + f8["bq"]) * scale
    wk_f = g2[:, None] * f8["wk"]
    bk_f = b2 @ f8["wk"] + f8["bk"]
    wv3 = f8["wv"].reshape(E, H, E)
    wv_f = np.ascontiguousarray((g2[:, None, None] * wv3).transpose(1, 0, 2))
    bvs = f8["bv"].reshape(H, E).sum(0) + b2 @ wv3.sum(axis=1)
    wi_f = g3[:, None] * f8["w_in"]
    bi_f = b3 @ f8["w_in"] + f8["b_in"]

    def ident_gate(g, b):
        return not (np.allclose(g, 1.0) and np.allclose(b, 0.0))

    aff = (ident_gate(g1, b1), ident_gate(g2, b2), ident_gate(g3, b3))
    import ml_dtypes

    FP8 = ml_dtypes.float8_e4m3

    def q8(a, s):
        return np.ascontiguousarray(np.clip(a * s, -240, 240), FP8)

    w = {
        "wq": q8(wq_f, S_Q), "bq": np.asarray(bq_f, np.float32),
        "wk": q8(wk_f, S_K), "bk": np.asarray(bk_f, np.float32),
        "wv": q8(wv_f, S_V),
        "bvs": np.asarray(bvs, ml_dtypes.bfloat16),
        "w_in": q8(wi_f, S_WI), "b_in": np.asarray(bi_f, np.float32),
        "w_out": q8(f8["w_out"], S_WO),
        "b_out": np.asarray(f8["b_out"] * S_WO, ml_dtypes.bfloat16),
    }
    for i, (g, b) in enumerate(((g1, b1), (g2, b2), (g3, b3))):
        if aff[i]:
            w[f"affg{i}"] = np.asarray(g, ml_dtypes.bfloat16)
            w[f"affb{i}"] = np.asarray(b, ml_dtypes.bfloat16)
    return w, aff


_NC_CACHE = {}


def _get_nc(T, E, H, aff):
    key = (T, E, H, aff)
    if key not in _NC_CACHE:
        nc = build(T, E, H, aff)
        nc.finalize()
        _NC_CACHE[key] = nc
    return _NC_CACHE[key]


def run(inputs, trace=False, tmpdir=None):
    from concourse.bass_utils import run_bass_kernel_spmd

    x = np.ascontiguousarray(np.asarray(inputs["x"], np.float32))
    y = np.ascontiguousarray(np.asarray(inputs["y"], np.float32))
    B, T, E = x.shape
    H = inputs["wv"].shape[1] // E
    assert B == N_CORES
    w, aff = host_prep(inputs, T, E, H)
    nc = _get_nc(T, E, H, aff)
    in_maps = [dict(w, x=x[c], y=y[c]) for c in range(B)]
    res = run_bass_kernel_spmd(
        nc, in_maps, core_ids=list(range(N_CORES)), trace=trace, tmpdir=tmpdir
    )
    o1 = np.stack([res.results[c]["o1"] for c in range(B)])
    oyn = np.stack([res.results[c]["oyn"] for c in range(B)])
    return (o1, oyn), res


def kernel(**inputs):
    (o1, oyn), _ = run(inputs)
    return (o1, oyn)


# revision 23
# speedup vs baseline: 1.2147x; 1.2147x over previous
"""Trainium2 Bass kernel for nn_Attention_33036888441230.

Cross-attention transformer block, B=8 batch sharded 1-per-core across 8
NeuronCores (pure data parallel, no collectives).

reference math (per batch):
  xn = LN(x,g1,b1); yn = LN(y,g2,b2)
  q = (xn@wq+bq).view(N,H,64); k = (yn@wk+bk).view(M,H,64)
  v = (yn@wv+bv).view(M,H,E)
  a = softmax(q.k^T/8, axis=m)
  dx = einsum('hnm,mhe->ne', a, v)       # heads summed
  h = LN3(xn + dx); out1 = h + relu(h@w_in+b_in)@w_out + b_out
  returns (out1, yn)

Implementation notes:
  - all large matmuls (q/k/v projections, attention.V, both MLP matmuls) run
    in fp8e4 with perf_mode=DoubleRow (2 fp8 weights per PE cell, ~2x rate);
    contraction pairs live in [P, 2, N] tiles (pair = two 128-deep k-subtiles
    adjacent in the free dim, matching the DoubleRow ISA layout).
  - weights are scaled by powers of two on host so fp8e4 sees ~unit-range
    values (TRN e4m3: max 240, min normal 2^-6); the inverse scales fold into
    the existing PSUM-evacuation activations (scale=) and, for the attention
    value path, into the ones-column (ones=32 == wv scale) so the softmax
    normalization ratio is exact and needs no extra ops.
  - softmax has no max-subtraction (scores provably tiny); row-sums come for
    free from a 32.0-column appended to the attention.V matmul; 1/rowsum is
    fused into the per-head accumulate (scalar_tensor_tensor).
  - softmax-of-rows sums to 1 => v-bias contributes sum_h bv_h; that row is
    broadcast once and pre-folded (together with the xn residual) into the
    attention accumulator during phase A, off the critical tail.
  - phase A interleaves per-tile layer-norms with projection matmul tiles so
    the PE never drains; the LN3 stats/normalize for the MLP run on ScalarE
    (Identity/Square + accum_out) woven under the last head pair's attention
    matmuls -- PE transposes are never interleaved into DoubleRow streams
    (they reload the PE weight registers and break double-buffering).
  - activations feature-major (PE transpose) for projections, token-major for
    attention-weighted sums; scores in bf16, normalization/residual in fp32.
  - score matmuls: heads in pairs, even head in PE rows 0:64, odd head in
    rows 64:128, interleaved so both run concurrently in the array.
"""

import sys

if "/opt/trn_rl_repo" not in sys.path:
    sys.path.insert(0, "/opt/trn_rl_repo")

from contextlib import ExitStack

import numpy as np

import concourse.bass as bass
import concourse.mybir as mybir
import concourse.tile as tile
from concourse import bacc
from concourse.masks import make_identity

F32 = mybir.dt.float32
BF16 = mybir.dt.bfloat16
F8 = mybir.dt.float8e4
OP = mybir.AluOpType
AF = mybir.ActivationFunctionType
DR = mybir.MatmulPerfMode.DoubleRow

P = 128
HD = 64
EPS = 1e-5
N_CORES = 8

# host-side power-of-two scales folded into fp8 weights
S_Q = 512.0   # wq (already /8 for attn scale) -> fp8
S_K = 64.0    # wk -> fp8
S_V = 32.0    # wv -> fp8; also the value of the row-sum ones-column
S_WI = 64.0   # w_in -> fp8
S_WO = 64.0   # w_out -> fp8 (and b_out pre-scaled by S_WO on host)

VP = 784      # v tile slot stride: 768 cols of v + ones col at 768, pad to 784


def _chunks(n, c=512):
    return [(i, min(i + c, n)) for i in range(0, n, c)]


def build(T, E, H, aff=(False, False, False)):
    """Build the per-core Bass graph. T tokens, E embed, H heads (HD=64)."""
    TT, ET = T // P, E // P
    NG = ET // 2   # contraction pair-groups for an E-deep reduction
    MG = TT // 2   # contraction pair-groups for a T-deep reduction
    HT = TT // 2   # token tiles per half
    assert H * HD == E
    nc = bacc.Bacc("TRN2", num_devices=N_CORES)

    x_d = nc.dram_tensor("x", [T, E], F32, kind="ExternalInput").ap()
    y_d = nc.dram_tensor("y", [T, E], F32, kind="ExternalInput").ap()
    wq_d = nc.dram_tensor("wq", [E, E], F8, kind="ExternalInput").ap()
    wk_d = nc.dram_tensor("wk", [E, E], F8, kind="ExternalInput").ap()
    wv_d = nc.dram_tensor("wv", [H, E, E], F8, kind="ExternalInput").ap()
    wi_d = nc.dram_tensor("w_in", [E, E], F8, kind="ExternalInput").ap()
    wo_d = nc.dram_tensor("w_out", [E, E], F8, kind="ExternalInput").ap()
    bq_d = nc.dram_tensor("bq", [E], F32, kind="ExternalInput").ap()
    bk_d = nc.dram_tensor("bk", [E], F32, kind="ExternalInput").ap()
    bvs_d = nc.dram_tensor("bvs", [E], BF16, kind="ExternalInput").ap()
    bi_d = nc.dram_tensor("b_in", [E], F32, kind="ExternalInput").ap()
    bo_d = nc.dram_tensor("b_out", [E], BF16, kind="ExternalInput").ap()
    aff_d = {}
    for i, need in enumerate(aff):
        if need:
            aff_d[i] = (
                nc.dram_tensor(f"affg{i}", [E], BF16, kind="ExternalInput").ap(),
                nc.dram_tensor(f"affb{i}", [E], BF16, kind="ExternalInput").ap(),
            )
    o1_d = nc.dram_tensor("o1", [T, E], F32, kind="ExternalOutput").ap()
    oyn_d = nc.dram_tensor("oyn", [T, E], F32, kind="ExternalOutput").ap()

    with tile.TileContext(nc) as tc, ExitStack() as ctx:
        persist = ctx.enter_context(tc.tile_pool(name="persist", bufs=1))
        ps_big = ctx.enter_context(tc.tile_pool(name="psb", bufs=2, space="PSUM"))
        stp = ctx.enter_context(tc.tile_pool(name="stats", bufs=8))

        xn = persist.tile([P, TT, E], F32, tag="xn")
        acc = persist.tile([P, TT, E], F32, tag="acc")
        ynT8 = persist.tile([P, ET, T], F8, tag="ynT8")
        BV = persist.tile([P, E], F32, tag="BV")
        ident = persist.tile([P, P], F32, tag="ident")
        ones_r = persist.tile([1, P], BF16, tag="ones")
        bq_sb = persist.tile([P, ET], F32, tag="bq")
        bk_sb = persist.tile([P, ET], F32, tag="bk")
        bi_sb = persist.tile([P, ET], F32, tag="bi")
        bo_row = persist.tile([1, E], BF16, tag="bo")
        bvs_row = persist.tile([1, E], BF16, tag="bvs")
        eps_col = persist.tile([P, 1], F32, tag="eps")
        iswo_col = persist.tile([P, 1], F32, tag="iswo")

        make_identity(nc, ident[:])
        nc.vector.memset(ones_r[:], 1.0)
        nc.vector.memset(eps_col[:], EPS)
        nc.vector.memset(iswo_col[:], 1.0 / S_WO)
        nc.sync.dma_start(bq_sb[:], bq_d.rearrange("(a p) -> p a", p=P))
        nc.sync.dma_start(bk_sb[:], bk_d.rearrange("(a p) -> p a", p=P))
        nc.sync.dma_start(bi_sb[:], bi_d.rearrange("(a p) -> p a", p=P))
        nc.sync.dma_start(bo_row[:], bo_d[None, :])
        nc.sync.dma_start(bvs_row[:], bvs_d[None, :])

        def bcast_row(row_ap, dst):  # (1,E) -> (128,E) via K=1 matmul
            ps = ps_big.tile([P, 1024], F32, tag="big")
            for c0, c1 in _chunks(E):
                nc.tensor.matmul(
                    ps[:, c0:c1], ones_r[:], row_ap[:, c0:c1],
                    start=True, stop=True,
                )
            nc.vector.tensor_copy(dst[:], ps[:, :E])

        aff_sb = {}
        for i, (gd, bd) in aff_d.items():
            g_row = persist.tile([1, E], BF16, tag=f"agr{i}")
            b_row = persist.tile([1, E], BF16, tag=f"abr{i}")
            nc.sync.dma_start(g_row[:], gd[None, :])
            nc.sync.dma_start(b_row[:], bd[None, :])
            g_t = persist.tile([P, E], F32, tag=f"ag{i}")
            b_t = persist.tile([P, E], F32, tag=f"ab{i}")
            bcast_row(g_row, g_t)
            bcast_row(b_row, b_t)
            aff_sb[i] = (g_t, b_t)

        # extra buffers for affine copies of residual-path tensors
        xn_res = xn
        if 0 in aff_sb:
            xn_res = persist.tile([P, TT, E], F32, tag="xna")

        qkp = ctx.enter_context(tc.tile_pool(name="qk", bufs=1))
        qT = qkp.tile([P, ET, T], BF16, tag="qT")
        kT = qkp.tile([P, ET, T], BF16, tag="kT")

        def layer_norm_tile(dst_ap, src_ap):
            """dst = (src - mean)/sqrt(var+eps), per-partition stats over E."""
            st6 = stp.tile([P, 2, 6], F32, tag="st6")
            half = src_ap.shape[-1] // 2
            nc.vector.bn_stats(st6[:, 0, :], src_ap[:, :half])
            nc.vector.bn_stats(st6[:, 1, :], src_ap[:, half:])
            mv = stp.tile([P, 2], F32, tag="mv")
            nc.vector.bn_aggr(mv[:], st6[:])
            std = stp.tile([P, 1], F32, tag="std")
            nc.scalar.activation(std[:], mv[:, 1:2], AF.Sqrt, bias=eps_col[:])
            rst = stp.tile([P, 1], F32, tag="rst")
            nc.vector.reciprocal(rst[:], std[:])
            nmr = stp.tile([P, 1], F32, tag="nmr")
            nc.vector.tensor_scalar(
                nmr[:], mv[:, 0:1], rst[:], -1.0, op0=OP.mult, op1=OP.mult
            )
            # big apply pass on ACT (idle at startup): x*rstd + (-mu*rstd)
            nc.scalar.activation(
                dst_ap, src_ap, AF.Identity, bias=nmr[:], scale=rst[:]
            )

        def affine_tile(dst_ap, src_ap, idx):
            g_t, b_t = aff_sb[idx]
            nc.vector.tensor_mul(dst_ap, src_ap, g_t[:])
            nc.vector.tensor_add(dst_ap, dst_ap, b_t[:])

        with tc.tile_pool(name="bc1", bufs=1) as bcp, \
             tc.tile_pool(name="io3", bufs=9) as iop, \
             tc.tile_pool(name="wpr", bufs=2) as wp, \
             tc.tile_pool(name="pstr", bufs=2, space="PSUM") as ps_tr:
            yn = bcp.tile([P, TT, E], F32, tag="yn")
            xnT8 = bcp.tile([P, ET, T], F8, tag="xnT8")

            # ~5us of junk matmuls on the identity tile: spins the PE HAM
            # activity window to full clock while the first input DMAs and
            # layer-norms run, so real matmuls start at 2.4 GHz
            warm = ps_tr.tile([P, 2, P], F32, tag="tr")
            for _ in range(48):
                nc.tensor.matmul(warm[:, 0, :], ident[:], ident[:],
                                 start=True, stop=True)

            # per-token-tile transpose into feature-major fp8 destination;
            # two 128x128 transposes share one PSUM bank -> one DVE evac
            def transpose_tile(dstT8, src3, tt):
                for eh in range(ET // 2):
                    pst = ps_tr.tile([P, 2, P], F32, tag="tr")
                    for k in range(2):
                        et = 2 * eh + k
                        nc.tensor.transpose(
                            pst[:, k, :], src3[:, tt, et * P:(et + 1) * P],
                            ident[:]
                        )
                    nc.vector.tensor_copy(
                        dstT8[:, 2 * eh:2 * eh + 2, tt * P:(tt + 1) * P], pst[:]
                    )

            def ln_y(tt, it):
                layer_norm_tile(yn[:, tt, :], it[:])
                if 1 in aff_sb:
                    ya = iop.tile([P, E], F32, tag="yaff")
                    affine_tile(ya[:], yn[:, tt, :], 1)
                    nc.gpsimd.dma_start(oyn_d[tt * P:(tt + 1) * P, :], ya[:])
                else:
                    nc.gpsimd.dma_start(
                        oyn_d[tt * P:(tt + 1) * P, :], yn[:, tt, :]
                    )
                transpose_tile(ynT8, yn, tt)

            def load_w8(w_d):
                wt = wp.tile([P, ET, E], F8, tag="w")
                nc.gpsimd.dma_start(
                    wt[:], w_d.rearrange("(a p) e -> p a e", p=P)
                )
                return wt

            # one output-feature tile of an fp8 DoubleRow projection
            def proj_mt(w8, b_sb, outT, srcT8, mt, c0, c1, scale):
                ps = ps_big.tile([P, 1024], F32, tag="big")
                for g in range(NG):
                    nc.tensor.matmul(
                        ps[:, : c1 - c0],
                        w8[:, 2 * g:2 * g + 2, mt * P:(mt + 1) * P],
                        srcT8[:, 2 * g:2 * g + 2, c0:c1],
                        start=(g == 0), stop=(g == NG - 1),
                        perf_mode=DR,
                    )
                nc.scalar.activation(
                    outT[:, mt, c0:c1], ps[:, : c1 - c0], AF.Identity,
                    bias=b_sb[:, mt:mt + 1], scale=scale,
                )

            # layer-norm x, then pre-fold the attention accumulator
            # acc = xn + sum_h bv_h, taking the residual add off the tail
            def ln_x(tt, it):
                layer_norm_tile(xn[:, tt, :], it[:])
                if 0 in aff_sb:
                    affine_tile(xn_res[:, tt, :], xn[:, tt, :], 0)
                nc.vector.tensor_add(acc[:, tt, :], xn_res[:, tt, :], BV[:])
                transpose_tile(xnT8, xn, tt)

            # weight DMAs + BV broadcast + all input-tile DMAs up front
            wk8 = load_w8(wk_d)
            wq8 = load_w8(wq_d)
            bcast_row(bvs_row, BV)
            ity, itx = [], []
            for tt in range(TT):
                ity.append(iop.tile([P, E], F32, tag="iny", name=f"iy{tt}"))
                nc.sync.dma_start(ity[tt][:], y_d[tt * P:(tt + 1) * P, :])
            for tt in range(TT):
                itx.append(iop.tile([P, E], F32, tag="inx", name=f"ix{tt}"))
                nc.sync.dma_start(itx[tt][:], x_d[tt * P:(tt + 1) * P, :])
            # interleave LN tiles with projection feature-tiles: each proj
            # mt is a clean 3-matmul DoubleRow burst that fills the PE while
            # ACT/DVE run the next tile's layer-norm
            for tt in range(HT):
                ln_y(tt, ity[tt])
            kh1 = [[0], [1], [2, 3], [4, 5]]
            for tt in range(HT, TT):
                ln_y(tt, ity[tt])
                for mt in kh1[tt - HT]:
                    proj_mt(wk8, bk_sb, kT, ynT8, mt, 0, T // 2, 1.0 / S_K)
            kh2 = [[0], [1], [2, 3], [4, 5]]
            for tt in range(HT):
                ln_x(tt, itx[tt])
                for mt in kh2[tt]:
                    proj_mt(wk8, bk_sb, kT, ynT8, mt, T // 2, T, 1.0 / S_K)
            qh1 = [[0, 1], [2, 3], [4], [5]]
            for tt in range(HT, TT):
                ln_x(tt, itx[tt])
                for mt in qh1[tt - HT]:
                    proj_mt(wq8, bq_sb, qT, xnT8, mt, 0, T // 2, 1.0 / S_Q)
            for mt in range(ET):
                proj_mt(wq8, bq_sb, qT, xnT8, mt, T // 2, T, 1.0 / S_Q)

        # --- attention head loop (LN3+MLP overlapped under the last pair) ---
        with tc.tile_pool(name="wvp", bufs=4) as wvp, \
             tc.tile_pool(name="expp", bufs=14) as expp, \
             tc.tile_pool(name="vp", bufs=14) as vpp, \
             tc.tile_pool(name="rcp", bufs=6) as rcp:
            sc_stack = ExitStack()
            ps_sc = sc_stack.enter_context(
                tc.tile_pool(name="pssc", bufs=2, space="PSUM")
            )

            # one head x one token-tile of attention.V (fp8 DoubleRow);
            # ps[:, E] = S_V * rowsum -> fused normalize-accumulate
            def t_nt(nt, expt, vt):
                ps = ps_big.tile([P, 1024], F32, tag="big")
                for g in range(MG):
                    for c0, c1 in _chunks(E + 1):
                        nc.tensor.matmul(
                            ps[:, c0:c1],
                            expt[g][:, :, nt * P:(nt + 1) * P],
                            vt[g][:, :, c0:c1],
                            start=(g == 0), stop=(g == MG - 1),
                            perf_mode=DR,
                        )
                rc = rcp.tile([P, 1], F32, tag="rc")
                nc.vector.reciprocal(rc[:], ps[:, E:E + 1])
                nc.vector.scalar_tensor_tensor(
                    acc[:, nt, :], ps[:, :E], rc[:], acc[:, nt, :],
                    op0=OP.mult, op1=OP.add,
                )

            def t_phase(h, expt, vt):
                for nt in range(TT):
                    t_nt(nt, expt, vt)

            # heads in pairs: even head uses PE rows 0:64, odd head rows
            # 64:128 -> interleaved score matmuls run concurrently in the
            # array (distinct row groups). v-projection (both heads, fp8
            # DoubleRow, one shared ynT8 weight-load per (mt, g)) is woven
            # between score mt-steps so PE never waits on the ACT exp drain.
            def pair_front(j):
                h0, h1 = 2 * j, 2 * j + 1
                wv8 = {}
                for h in (h0, h1):
                    wt = wvp.tile([P, ET, E], F8, tag="wv")
                    nc.gpsimd.dma_start(
                        wt[:], wv_d[h].rearrange("(a p) e -> p a e", p=P)
                    )
                    wv8[h] = wt
                expt = {h0: [], h1: []}
                vt = {h0: [], h1: []}
                for mt in range(TT):
                    sl = mt % 2
                    if sl == 0:
                        for h in (h0, h1):
                            ex = expp.tile([P, 2, T], F8, tag="exp")
                            expt[h].append(ex)
                            v_ = vpp.tile([P, 2, VP], F8, tag="v")
                            nc.gpsimd.memset(v_[:, :, E:E + 1], S_V)
                            vt[h].append(v_)
                    ps0 = ps_sc.tile([P, 1024], F32, tag="sc")
                    ps1 = ps_sc.tile([P, 1024], F32, tag="sc")
                    for c0, c1 in _chunks(T):
                        nc.tensor.matmul(
                            ps0[:, c0:c1],
                            kT[0:HD, j, mt * P:(mt + 1) * P],
                            qT[0:HD, j, c0:c1],
                            start=True, stop=True,
                        )
                        nc.tensor.matmul(
                            ps1[:, c0:c1],
                            kT[HD:P, j, mt * P:(mt + 1) * P],
                            qT[HD:P, j, c0:c1],
                            start=True, stop=True,
                        )
                    for h, ps in ((h0, ps0), (h1, ps1)):
                        nc.scalar.activation(
                            expt[h][mt // 2][:, sl, :], ps[:, :T], AF.Exp
                        )
                    # v-projection for this m-tile, both heads, one shared
                    # stationary (ynT8 tokens) per contraction pair-group
                    psv0 = ps_big.tile([P, 1024], F32, tag="big")
                    psv1 = ps_big.tile([P, 1024], F32, tag="big")
                    for g in range(NG):
                        lhs = ynT8[:, 2 * g:2 * g + 2, mt * P:(mt + 1) * P]
                        for c0, c1 in _chunks(E):
                            nc.tensor.matmul(
                                psv0[:, c0:c1], lhs,
                                wv8[h0][:, 2 * g:2 * g + 2, c0:c1],
                                start=(g == 0), stop=(g == NG - 1),
                                perf_mode=DR,
                            )
                            nc.tensor.matmul(
                                psv1[:, c0:c1], lhs,
                                wv8[h1][:, 2 * g:2 * g + 2, c0:c1],
                                start=(g == 0), stop=(g == NG - 1),
                                perf_mode=DR,
                            )
                    nc.vector.tensor_copy(vt[h0][mt // 2][:, sl, :E], psv0[:, :E])
                    nc.vector.tensor_copy(vt[h1][mt // 2][:, sl, :E], psv1[:, :E])
                return expt, vt

            for j in range(H // 2 - 1):
                h0, h1 = 2 * j, 2 * j + 1
                expt, vt = pair_front(j)
                t_phase(h0, expt[h0], vt[h0])
                t_phase(h1, expt[h1], vt[h1])
            hl0, hl1 = H - 2, H - 1
            expt, vt = pair_front(H // 2 - 1)
            sc_stack.close()

            # --- residual + LN3 + MLP (fp8 DoubleRow) ---
            with tc.tile_pool(name="mlp", bufs=1) as mp, \
                 tc.tile_pool(name="out3", bufs=3) as op_, \
                 tc.tile_pool(name="scrp", bufs=2) as scrp, \
                 tc.tile_pool(name="pstr2", bufs=2, space="PSUM") as ps_tr:
                hT8 = mp.tile([P, ET, T], F8, tag="hT8")
                ru8T = mp.tile([P, ET, T], F8, tag="ru8T")
                wi8 = mp.tile([P, ET, E], F8, tag="wi")
                wo8 = mp.tile([P, ET, E], F8, tag="wo")
                h_res = acc
                if 2 in aff_sb:
                    h_res = mp.tile([P, TT, E], F32, tag="ha")
                nc.gpsimd.dma_start(
                    wi8[:], wi_d.rearrange("(a p) l -> p a l", p=P)
                )
                nc.gpsimd.dma_start(
                    wo8[:], wo_d.rearrange("(a p) l -> p a l", p=P)
                )

                # transposes of normalized h into hT8 (PE, batched cleanly
                # outside the DoubleRow streams)
                def transpose_h(nt):
                    for eh in range(ET // 2):
                        pst = ps_tr.tile([P, 2, P], F32, tag="tr")
                        for k in range(2):
                            et = 2 * eh + k
                            nc.tensor.transpose(
                                pst[:, k, :], acc[:, nt, et * P:(et + 1) * P],
                                ident[:]
                            )
                        nc.vector.tensor_copy(
                            hT8[:, 2 * eh:2 * eh + 2, nt * P:(nt + 1) * P],
                            pst[:]
                        )

                # LN3 stats + normalize entirely on ScalarE/few tiny DVE ops
                # (no PE work, no bn_stats -- DVE is busy with the attention
                # evacuations in this window); acc normalized in place
                def ln3(nt):
                    s1 = stp.tile([P, 1], F32, tag="s1")
                    sq = stp.tile([P, 1], F32, tag="sq")
                    scr = scrp.tile([P, E], F32, tag="scr")
                    nc.scalar.activation(
                        scr[:], acc[:, nt, :], AF.Identity, accum_out=s1[:]
                    )
                    nc.scalar.activation(
                        scr[:], acc[:, nt, :], AF.Square, accum_out=sq[:]
                    )
                    m2 = stp.tile([P, 1], F32, tag="m2")
                    nc.scalar.activation(m2[:], s1[:], AF.Square, scale=1.0 / E)
                    vpe = stp.tile([P, 1], F32, tag="vpe")
                    nc.vector.tensor_scalar(
                        vpe[:], sq[:], 1.0 / E, m2[:],
                        op0=OP.mult, op1=OP.subtract,
                    )
                    std = stp.tile([P, 1], F32, tag="std")
                    nc.scalar.activation(std[:], vpe[:], AF.Sqrt, bias=eps_col[:])
                    rst = stp.tile([P, 1], F32, tag="rst")
                    nc.vector.reciprocal(rst[:], std[:])
                    nmr = stp.tile([P, 1], F32, tag="nmr")
                    nc.vector.tensor_scalar(
                        nmr[:], s1[:], rst[:], -1.0 / E, op0=OP.mult, op1=OP.mult
                    )
                    nc.scalar.activation(
                        acc[:, nt, :], acc[:, nt, :], AF.Identity,
                        bias=nmr[:], scale=rst[:],
                    )
                    if 2 in aff_sb:
                        affine_tile(h_res[:, nt, :], acc[:, nt, :], 2)

                # last pair: both heads' attention interleaved per token
                # tile, with the LN3 chain hidden under the PE matmuls
                for nt in range(TT):
                    t_nt(nt, expt[hl0], vt[hl0])
                    t_nt(nt, expt[hl1], vt[hl1])
                    ln3(nt)

                def u_chunk(c0, c1):
                    # u^T = relu(w_in^T @ hT8 / S_WI + b_in), token columns
                    for mt in range(ET):
                        ps = ps_big.tile([P, 1024], F32, tag="big")
                        for g in range(NG):
                            nc.tensor.matmul(
                                ps[:, : c1 - c0],
                                wi8[:, 2 * g:2 * g + 2, mt * P:(mt + 1) * P],
                                hT8[:, 2 * g:2 * g + 2, c0:c1],
                                start=(g == 0), stop=(g == NG - 1),
                                perf_mode=DR,
                            )
                        nc.scalar.activation(
                            ru8T[:, mt, c0:c1], ps[:, : c1 - c0], AF.Relu,
                            bias=bi_sb[:, mt:mt + 1], scale=1.0 / S_WI,
                        )

                def out_tile(nt):
                    # out1 = (ru8T^T @ wo8 + S_WO*b_out)/S_WO + h
                    ps = ps_big.tile([P, 1024], F32, tag="big")
                    for g in range(NG):
                        for c0, c1 in _chunks(E):
                            nc.tensor.matmul(
                                ps[:, c0:c1],
                                ru8T[:, 2 * g:2 * g + 2, nt * P:(nt + 1) * P],
                                wo8[:, 2 * g:2 * g + 2, c0:c1],
                                start=(g == 0), stop=False,
                                perf_mode=DR,
                            )
                    for c0, c1 in _chunks(E):
                        nc.tensor.matmul(
                            ps[:, c0:c1], ones_r[:], bo_row[:, c0:c1],
                            start=False, stop=True,
                        )
                    ot = op_.tile([P, E], F32, tag="ot")
                    nc.vector.scalar_tensor_tensor(
                        ot[:], ps[:, :E], iswo_col[:], h_res[:, nt, :],
                        op0=OP.mult, op1=OP.add,
                    )
                    nc.gpsimd.dma_start(o1_d[nt * P:(nt + 1) * P, :], ot[:])

                # batch the LN3 transposes per token-half (clean PE bursts,
                # never interleaved with DoubleRow matmuls), then the MLP
                for nt in range(TT // 2):
                    transpose_h(nt)
                u_chunk(0, T // 2)
                for nt in range(TT // 2, TT):
                    transpose_h(nt)
                for nt in range(TT // 2):
                    out_tile(nt)
                u_chunk(T // 2, T)
                for nt in range(TT // 2, TT):
                    out_tile(nt)

    return nc


def host_prep(inputs, T, E, H):
    """Fold LN affines / scale / v-bias into weights (float64 on host)."""
    f8 = {k: np.asarray(v, np.float64) for k, v in inputs.items()}
    g1, b1 = f8["ln1_g"], f8["ln1_b"]
    g2, b2 = f8["ln2_g"], f8["ln2_b"]
    g3, b3 = f8["ln3_g"], f8["ln3_b"]
    scale = 1.0 / np.sqrt(HD)
    wq_f = (g1[:, None] * f8["wq"]) * scale
    bq_f = (b1 @ f8["wq"] + f8["bq"]) * scale
    wk_f = g2[:, None] * f8["wk"]
    bk_f = b2 @ f8["wk"] + f8["bk"]
    wv3 = f8["wv"].reshape(E, H, E)
    wv_f = np.ascontiguousarray((g2[:, None, None] * wv3).transpose(1, 0, 2))
    bvs = f8["bv"].reshape(H, E).sum(0) + b2 @ wv3.sum(axis=1)
    wi_f = g3[:, None] * f8["w_in"]
    bi_f = b3 @ f8["w_in"] + f8["b_in"]

    def ident_gate(g, b):
        return not (np.allclose(g, 1.0) and np.allclose(b, 0.0))

    aff = (ident_gate(g1, b1), ident_gate(g2, b2), ident_gate(g3, b3))
    import ml_dtypes

    FP8 = ml_dtypes.float8_e4m3

    def q8(a, s):
        return np.ascontiguousarray(np.clip(a * s, -240, 240), FP8)

    w = {
        "wq": q8(wq_f, S_Q), "bq": np.asarray(bq_f, np.float32),
        "wk": q8(wk_f, S_K), "bk": np.asarray(bk_f, np.float32),
        "wv": q8(wv_f, S_V),
        "bvs": np.asarray(bvs, ml_dtypes.bfloat16),
        "w_in": q8(wi_f, S_WI), "b_in": np.asarray(bi_f, np.float32),
        "w_out": q8(f8["w_out"], S_WO),
        "b_out": np.asarray(f8["b_out"] * S_WO, ml_dtypes.bfloat16),
    }
    for i, (g, b) in enumerate(((g1, b1), (g2, b2), (g3, b3))):
        if aff[i]:
            w[f"affg{i}"] = np.asarray(g, ml_dtypes.bfloat16)
            w[f"affb{i}"] = np.asarray(b, ml_dtypes.bfloat16)
    return w, aff


_NC_CACHE = {}


def _get_nc(T, E, H, aff):
    key = (T, E, H, aff)
    if key not in _NC_CACHE:
        nc = build(T, E, H, aff)
        nc.finalize()
        _NC_CACHE[key] = nc
    return _NC_CACHE[key]


def run(inputs, trace=False, tmpdir=None):
    from concourse.bass_utils import run_bass_kernel_spmd

    x = np.ascontiguousarray(np.asarray(inputs["x"], np.float32))
    y = np.ascontiguousarray(np.asarray(inputs["y"], np.float32))
    B, T, E = x.shape
    H = inputs["wv"].shape[1] // E
    assert B == N_CORES
    w, aff = host_prep(inputs, T, E, H)
    nc = _get_nc(T, E, H, aff)
    in_maps = [dict(w, x=x[c], y=y[c]) for c in range(B)]
    res = run_bass_kernel_spmd(
        nc, in_maps, core_ids=list(range(N_CORES)), trace=trace, tmpdir=tmpdir
    )
    o1 = np.stack([res.results[c]["o1"] for c in range(B)])
    oyn = np.stack([res.results[c]["oyn"] for c in range(B)])
    return (o1, oyn), res


def kernel(**inputs):
    (o1, oyn), _ = run(inputs)
    return (o1, oyn)
